# revision 1
# baseline (speedup 1.0000x reference)
"""GQA with RoPE, tanh soft-cap, symmetric sliding-window mask — 8-core trn2.

Sharding: TP4 (head groups of 4 q-heads / 2 kv heads) x DP2 (batch).
Core c: dp = c // 4 (batch index), tp = c % 4 (head group).
Each core computes a partial o_proj product for its batch; host sums the
4 partials per batch (row-parallel o_proj reduced on host).

Layouts on device (all matmul operands bf16, accumulation fp32):
  xT   [3584, 2048]  hidden[dp].T           (contraction dim on partitions)
  Q^T  [1024, 2048]  per-core q-head cols x tokens
  K^T  [ 512, 2048]
  V    [2048,  512]  natural (tokens on partitions)
  S^T  [k, q] blocks -> softcap/exp -> E^T; PV gives out^T [dv, q];
  denominators via ones-matmul (partition reduce); o_proj consumes out^T.
"""

import math
import numpy as np

H, KV, D = 16, 8, 256
EMBED = 3584
B, S = 2, 2048
SOFT_CAP = 50.0
WINDOW = 1024
SCALE = 1.0 / 16.0  # 1/sqrt(D)

NE = EMBED // 128          # 28 embed chunks
QCOLS = 1024               # per-core q cols (4 heads)
KCOLS = 512                # per-core kv cols (2 heads)
NTOKB = 4                  # 512-token blocks per batch
NKB = S // 128             # 16 k blocks

# ---- sliding-window block schedule (q-blocks of 512, k-blocks of 128) ----
# delta = q0 - k0; block live iff exists |i-j| <= WINDOW; partial iff any
# element masked.  mask[kk, qq] additive: 0 if |delta + qq - kk| <= WINDOW
# else -2e7 (multiplied by exp scale 50 -> -1e9).
_DELTAS = [-1024, -1152, -1280, -1408, 640, 768, 896, 1024]


def _block_schedule():
    sched = []  # per qb: list of (kb, mask_idx or None)
    for qb in range(NTOKB):
        q0 = qb * 512
        row = []
        for kb in range(NKB):
            k0 = kb * 128
            if k0 > q0 + 511 + WINDOW or k0 + 127 < q0 - WINDOW:
                continue  # fully masked
            if k0 < q0 - 513 or k0 > q0 + 897:
                d = q0 - k0
                row.append((kb, _DELTAS.index(d)))
            else:
                row.append((kb, None))
        sched.append(row)
    return sched


_SCHED = _block_schedule()

_NC_CACHE = {}


def _build_nc():
    if "nc" in _NC_CACHE:
        return _NC_CACHE["nc"]
    from contextlib import ExitStack
    from concourse import bass, mybir, tile
    from concourse.vector_clock import ScopedClock
    from bass_rust import SyncInfo

    # This walrus build only accepts a single sync-wait command on SP ctrl
    # instructions; split the tail-drain waits across one nop each.
    def _patched_drain_and_barrier(self, tick_clock, wait_clock):
        nc = self.nc
        probe = nc.sync.nop(nofuse=True)
        wait_clock.add_sem_waits(probe.ins, ScopedClock({None: tick_clock.global_clock}))
        si = probe.ins.sync_info
        waits = list(si.on_wait)
        probe.ins.sync_info = SyncInfo(on_wait=waits[:1], on_update=list(si.on_update))
        for i in range(1, len(waits)):
            ni = nc.sync.nop(nofuse=True)
            ni.ins.sync_info = SyncInfo(on_wait=waits[i : i + 1], on_update=[])
        nc.sync.drain()
        nc.all_engine_barrier()
        popped = nc._tile_sem_poison_stack.pop()
        assert popped is self._sem_poison
        nc.clear_and_free_semaphores(list(self.sems.allocated().values()))
        nc.all_engine_barrier()

    tile.TileContext._drain_and_barrier = _patched_drain_and_barrier

    # Same walrus limitation applies to every instruction: at most one sync
    # wait command.  Split extra waits onto nops on the same engine placed
    # immediately before the instruction (engine program order preserved).
    if not getattr(tile.TileContext, "_split_waits_patched", False):
        _orig_lower = tile.TileContext._lower_ordered_insts

        def _patched_lower(self, ordered):
            cnt = [0]
            for bname, insts in list(ordered.items()):
                newl = []
                for inst in insts:
                    try:
                        si = inst.sync_info
                        w = list(si.on_wait)
                    except Exception:
                        w = []
                    if len(w) > 1:
                        for wx in w[:-1]:
                            nop = mybir.InstNoOp(
                                name=f"TSWN{cnt[0]}",
                                engine=inst.engine,
                                ins=[],
                                outs=[],
                                sync_info=SyncInfo(on_wait=[wx], on_update=[]),
                            )
                            cnt[0] += 1
                            self.nc.register_instruction(nop, overwrite=True)
                            newl.append(nop)
                        inst.sync_info = SyncInfo(
                            on_wait=[w[-1]], on_update=list(si.on_update)
                        )
                    newl.append(inst)
                ordered[bname] = newl
            return _orig_lower(self, ordered)

        tile.TileContext._lower_ordered_insts = _patched_lower
        tile.TileContext._split_waits_patched = True

    dt = mybir.dt
    AF = mybir.ActivationFunctionType
    OP = mybir.AluOpType

    nc = bass.Bass()
    xT = nc.dram_tensor("xT", [EMBED, S], dt.bfloat16, kind="ExternalInput")
    wq = nc.dram_tensor("wq", [EMBED, QCOLS], dt.bfloat16, kind="ExternalInput")
    wk = nc.dram_tensor("wk", [EMBED, KCOLS], dt.bfloat16, kind="ExternalInput")
    wv = nc.dram_tensor("wv", [EMBED, KCOLS], dt.bfloat16, kind="ExternalInput")
    wo = nc.dram_tensor("wo", [QCOLS, EMBED], dt.bfloat16, kind="ExternalInput")
    cosT = nc.dram_tensor("cosT", [128, S], dt.bfloat16, kind="ExternalInput")
    sinT = nc.dram_tensor("sinT", [128, S], dt.bfloat16, kind="ExternalInput")
    masks = nc.dram_tensor("masks", [len(_DELTAS), 128, 512], dt.bfloat16, kind="ExternalInput")
    y = nc.dram_tensor("y", [S, EMBED], dt.float32, kind="ExternalOutput")

    wq_r = wq.rearrange("(a p) m -> p a m", p=128)   # [128, 28, 1024]
    wk_r = wk.rearrange("(a p) m -> p a m", p=128)   # [128, 28, 512]
    wv_r = wv.rearrange("(a p) m -> p a m", p=128)
    wo_r = wo.rearrange("(a p) m -> p a m", p=128)   # [128, 8, 3584]

    with tile.TileContext(nc) as tc, ExitStack() as top:
        persist = top.enter_context(tc.tile_pool(name="persist", bufs=1))
        q_sb = persist.tile([128, 8, S], dt.bfloat16, tag="q")     # Q^T
        k_sb = persist.tile([128, 4, S], dt.bfloat16, tag="k")     # K^T
        v_sb = persist.tile([128, NKB, 512], dt.bfloat16, tag="v")  # V natural
        o_sb = persist.tile([128, 8, S], dt.bfloat16, tag="o")     # out^T
        cos_sb = persist.tile([128, S], dt.bfloat16, tag="cos")
        sin_sb = persist.tile([128, S], dt.bfloat16, tag="sin")
        mask_sb = persist.tile([128, len(_DELTAS), 512], dt.bfloat16, tag="mask")
        ones_col = persist.tile([128, 1], dt.bfloat16, tag="ones_col")   # denom lhsT
        ones_row = persist.tile([1, 128], dt.float32, tag="ones_row")    # bcast lhsT

        nc.sync.dma_start(cos_sb[:], cosT[:])
        nc.sync.dma_start(sin_sb[:], sinT[:])
        for i in range(len(_DELTAS)):
            nc.sync.dma_start(mask_sb[:, i, :], masks[i])
        nc.vector.memset(ones_col[:], 1.0)
        nc.vector.memset(ones_row[:], 1.0)

        # ---------------- Phase 1a: Q^T = (Wq^T x) with RoPE ----------------
        with ExitStack() as ph:
            wq_pool = ph.enter_context(tc.tile_pool(name="wq", bufs=1))
            xt_pool = ph.enter_context(tc.tile_pool(name="xt", bufs=4))
            ps_pool = ph.enter_context(
                tc.tile_pool(name="ps1", bufs=1, space="PSUM")
            )
            tmp_pool = ph.enter_context(tc.tile_pool(name="rtmp", bufs=4))

            wq_t = wq_pool.tile([128, NE, QCOLS], dt.bfloat16, tag="wq")
            for e in range(NE):
                nc.sync.dma_start(wq_t[:, e, :], wq_r[:, e, :])

            for g in range(NTOKB):
                tsl = slice(g * 512, (g + 1) * 512)
                ps = [ps_pool.tile([128, 512], dt.float32, name=f"psq{i}", tag=f"psq{i}") for i in range(8)]
                for e in range(NE):
                    xt = xt_pool.tile([128, 512], dt.bfloat16, tag="xt")
                    nc.sync.dma_start(xt[:], xT[e * 128 : (e + 1) * 128, tsl])
                    for qrb in range(8):
                        nc.tensor.matmul(
                            ps[qrb][:],
                            wq_t[:, e, qrb * 128 : (qrb + 1) * 128],
                            xt[:],
                            start=(e == 0),
                            stop=(e == NE - 1),
                        )
                # RoPE per head pair, write bf16 Q^T
                for h in range(4):
                    lo, hi = ps[2 * h], ps[2 * h + 1]
                    t1 = tmp_pool.tile([128, 512], dt.float32, tag="t1")
                    t2 = tmp_pool.tile([128, 512], dt.float32, tag="t2")
                    nc.vector.tensor_tensor(t1[:], lo[:], cos_sb[:, tsl], OP.mult)
                    nc.vector.tensor_tensor(t2[:], hi[:], sin_sb[:, tsl], OP.mult)
                    nc.vector.tensor_tensor(q_sb[:, 2 * h, tsl], t1[:], t2[:], OP.subtract)
                    t3 = tmp_pool.tile([128, 512], dt.float32, tag="t1")
                    t4 = tmp_pool.tile([128, 512], dt.float32, tag="t2")
                    nc.vector.tensor_tensor(t3[:], hi[:], cos_sb[:, tsl], OP.mult)
                    nc.vector.tensor_tensor(t4[:], lo[:], sin_sb[:, tsl], OP.mult)
                    nc.vector.tensor_tensor(q_sb[:, 2 * h + 1, tsl], t3[:], t4[:], OP.add)

        # ---------------- Phase 1b: K^T (RoPE) and V ----------------
        with ExitStack() as ph:
            wk_pool = ph.enter_context(tc.tile_pool(name="wkv", bufs=1))
            xt_pool = ph.enter_context(tc.tile_pool(name="xt2", bufs=4))
            ps_pool = ph.enter_context(
                tc.tile_pool(name="ps2", bufs=1, space="PSUM")
            )
            tmp_pool = ph.enter_context(tc.tile_pool(name="rtmp2", bufs=4))

            wk_t = wk_pool.tile([128, NE, KCOLS], dt.bfloat16, tag="wk")
            wv_t = wk_pool.tile([128, NE, KCOLS], dt.bfloat16, tag="wv")
            for e in range(NE):
                nc.sync.dma_start(wk_t[:, e, :], wk_r[:, e, :])
                nc.sync.dma_start(wv_t[:, e, :], wv_r[:, e, :])

            for g in range(NTOKB):
                tsl = slice(g * 512, (g + 1) * 512)
                psk = [ps_pool.tile([128, 512], dt.float32, name=f"psk{i}", tag=f"psk{i}") for i in range(4)]
                psv = [ps_pool.tile([128, 512], dt.float32, name=f"psv{i}", tag=f"psv{i}") for i in range(4)]
                for e in range(NE):
                    xt = xt_pool.tile([128, 512], dt.bfloat16, tag="xt")
                    nc.sync.dma_start(xt[:], xT[e * 128 : (e + 1) * 128, tsl])
                    for krb in range(4):
                        nc.tensor.matmul(
                            psk[krb][:],
                            wk_t[:, e, krb * 128 : (krb + 1) * 128],
                            xt[:],
                            start=(e == 0),
                            stop=(e == NE - 1),
                        )
                    for ts_ in range(4):
                        nc.tensor.matmul(
                            psv[ts_][:],
                            xt[:, ts_ * 128 : (ts_ + 1) * 128],
                            wv_t[:, e, :],
                            start=(e == 0),
                            stop=(e == NE - 1),
                        )
                for h in range(2):
                    lo, hi = psk[2 * h], psk[2 * h + 1]
                    t1 = tmp_pool.tile([128, 512], dt.float32, tag="t1")
                    t2 = tmp_pool.tile([128, 512], dt.float32, tag="t2")
                    nc.vector.tensor_tensor(t1[:], lo[:], cos_sb[:, tsl], OP.mult)
                    nc.vector.tensor_tensor(t2[:], hi[:], sin_sb[:, tsl], OP.mult)
                    nc.vector.tensor_tensor(k_sb[:, 2 * h, tsl], t1[:], t2[:], OP.subtract)
                    t3 = tmp_pool.tile([128, 512], dt.float32, tag="t1")
                    t4 = tmp_pool.tile([128, 512], dt.float32, tag="t2")
                    nc.vector.tensor_tensor(t3[:], hi[:], cos_sb[:, tsl], OP.mult)
                    nc.vector.tensor_tensor(t4[:], lo[:], sin_sb[:, tsl], OP.mult)
                    nc.vector.tensor_tensor(k_sb[:, 2 * h + 1, tsl], t3[:], t4[:], OP.add)
                for ts_ in range(4):
                    nc.scalar.copy(v_sb[:, g * 4 + ts_, :], psv[ts_][:])

        # ---------------- Phase 2: attention ----------------
        with ExitStack() as ph:
            s_pool = ph.enter_context(tc.tile_pool(name="spsum", bufs=4, space="PSUM"))
            o_pool = ph.enter_context(tc.tile_pool(name="opsum", bufs=1, space="PSUM"))
            d_pool = ph.enter_context(tc.tile_pool(name="dpsum", bufs=1, space="PSUM"))
            b_pool = ph.enter_context(tc.tile_pool(name="bpsum", bufs=1, space="PSUM"))
            w_pool = ph.enter_context(tc.tile_pool(name="work", bufs=3))
            r_pool = ph.enter_context(tc.tile_pool(name="rwork", bufs=2))

            for h in range(4):
                gh = h // 2  # local kv head
                for qb in range(NTOKB):
                    qsl = slice(qb * 512, (qb + 1) * 512)
                    o_lo = o_pool.tile([128, 512], dt.float32, tag="olo")
                    o_hi = o_pool.tile([128, 512], dt.float32, tag="ohi")
                    dn = d_pool.tile([1, 512], dt.float32, tag="dn")
                    blocks = _SCHED[qb]
                    nblk = len(blocks)
                    for bi, (kb, mi) in enumerate(blocks):
                        ksl = slice(kb * 128, (kb + 1) * 128)
                        st = s_pool.tile([128, 512], dt.float32, tag="st")
                        for dc in range(2):
                            nc.tensor.matmul(
                                st[:],
                                k_sb[:, 2 * gh + dc, ksl],
                                q_sb[:, 2 * h + dc, qsl],
                                start=(dc == 0),
                                stop=(dc == 1),
                            )
                        tt = w_pool.tile([128, 512], dt.float32, tag="tt")
                        nc.scalar.activation(tt[:], st[:], AF.Tanh, scale=SCALE / SOFT_CAP)
                        if mi is not None:
                            nc.vector.tensor_tensor(tt[:], tt[:], mask_sb[:, mi, :], OP.add)
                        et = w_pool.tile([128, 512], dt.bfloat16, tag="et")
                        nc.scalar.activation(et[:], tt[:], AF.Exp, scale=SOFT_CAP)
                        first, last = bi == 0, bi == nblk - 1
                        nc.tensor.matmul(
                            o_lo[:],
                            v_sb[:, kb, 256 * gh : 256 * gh + 128],
                            et[:],
                            start=first,
                            stop=last,
                        )
                        nc.tensor.matmul(
                            o_hi[:],
                            v_sb[:, kb, 256 * gh + 128 : 256 * gh + 256],
                            et[:],
                            start=first,
                            stop=last,
                        )
                        nc.tensor.matmul(
                            dn[:], ones_col[:], et[:], start=first, stop=last
                        )
                    recip = r_pool.tile([1, 512], dt.float32, tag="recip")
                    nc.vector.reciprocal(recip[:], dn[:])
                    bc = b_pool.tile([128, 512], dt.float32, tag="bc")
                    nc.tensor.matmul(bc[:], ones_row[:], recip[:], start=True, stop=True)
                    rb = r_pool.tile([128, 512], dt.float32, tag="rb")
                    nc.scalar.copy(rb[:], bc[:])
                    nc.vector.tensor_tensor(o_sb[:, 2 * h, qsl], o_lo[:], rb[:], OP.mult)
                    nc.vector.tensor_tensor(o_sb[:, 2 * h + 1, qsl], o_hi[:], rb[:], OP.mult)

        # ---------------- Phase 3: o_proj partial ----------------
        with ExitStack() as ph:
            wo_pool = ph.enter_context(tc.tile_pool(name="wo", bufs=1))
            y_pool = ph.enter_context(tc.tile_pool(name="ypsum", bufs=3, space="PSUM"))
            ys_pool = ph.enter_context(tc.tile_pool(name="ystage", bufs=3))

            wo_t = wo_pool.tile([128, 8, EMBED], dt.bfloat16, tag="wo")
            for c in range(8):
                nc.sync.dma_start(wo_t[:, c, :], wo_r[:, c, :])

            for tb in range(S // 128):
                tsl = slice(tb * 128, (tb + 1) * 128)
                for eb in range(EMBED // 512):
                    esl = slice(eb * 512, (eb + 1) * 512)
                    psy = y_pool.tile([128, 512], dt.float32, tag="psy")
                    for c in range(8):
                        nc.tensor.matmul(
                            psy[:],
                            o_sb[:, c, tsl],
                            wo_t[:, c, esl],
                            start=(c == 0),
                            stop=(c == 7),
                        )
                    yst = ys_pool.tile([128, 512], dt.float32, tag="yst")
                    nc.scalar.copy(yst[:], psy[:])
                    nc.sync.dma_start(y[tsl, esl], yst[:])

    _NC_CACHE["nc"] = nc
    return nc


def _host_inputs(hidden_states, Wq, Wk, Wv, Wo):
    import ml_dtypes

    bf16 = ml_dtypes.bfloat16
    # rope tables (match reference fp32 math)
    inv_freq = 1.0 / (10000.0 ** (np.arange(0, D, 2, dtype=np.float32) / D))
    pos = np.arange(S, dtype=np.float32)
    freqs = np.outer(inv_freq, pos)  # [128, S]  (transposed table)
    cosT = np.cos(freqs).astype(bf16)
    sinT = np.sin(freqs).astype(bf16)

    # additive window masks (scaled by 1/SOFT_CAP; exp scale multiplies back)
    kk = np.arange(128)[:, None]
    qq = np.arange(512)[None, :]
    m = np.stack(
        [
            np.where(np.abs(d + qq - kk) <= WINDOW, 0.0, -2.0e7).astype(np.float32)
            for d in _DELTAS
        ]
    ).astype(bf16)

    xT = [np.ascontiguousarray(hidden_states[b].T).astype(bf16) for b in range(B)]
    wq_s = [np.ascontiguousarray(Wq[:, t * 1024 : (t + 1) * 1024]).astype(bf16) for t in range(4)]
    wk_s = [np.ascontiguousarray(Wk[:, t * 512 : (t + 1) * 512]).astype(bf16) for t in range(4)]
    wv_s = [np.ascontiguousarray(Wv[:, t * 512 : (t + 1) * 512]).astype(bf16) for t in range(4)]
    wo_s = [np.ascontiguousarray(Wo[t * 1024 : (t + 1) * 1024, :]).astype(bf16) for t in range(4)]

    in_maps = []
    for c in range(8):
        dp, tp = c // 4, c % 4
        in_maps.append(
            {
                "xT": xT[dp],
                "wq": wq_s[tp],
                "wk": wk_s[tp],
                "wv": wv_s[tp],
                "wo": wo_s[tp],
                "cosT": cosT,
                "sinT": sinT,
                "masks": m,
            }
        )
    return in_maps


def kernel(hidden_states, Wq, Wk, Wv, Wo, _trace=False, _trace_kwargs=None):
    from concourse.bass_utils import run_bass_kernel_spmd

    nc = _build_nc()
    in_maps = _host_inputs(hidden_states, Wq, Wk, Wv, Wo)
    res = run_bass_kernel_spmd(
        nc, in_maps, core_ids=list(range(8)), trace=_trace, **(_trace_kwargs or {})
    )
    out = np.zeros((B, S, EMBED), np.float32)
    for c in range(8):
        out[c // 4] += res.results[c]["y"]
    if _trace:
        kernel._last = res
    return out

